# revision 38
# baseline (speedup 1.0000x reference)
"""MoE (16 routed experts, top-2, + shared expert) for 8 Trainium2 cores.

Sharding strategy (expert-parallel, per the hint):
  - Each core owns 2 routed experts. The host computes the gate routing
    (dispatch = sharding: selecting which token rows go to which core) and
    ships each core a gathered, capacity-padded token block per expert.
  - The gate combine WEIGHTS are recomputed on device (fp32 matmul +
    softmax on the gathered tokens) so all output values are
    device-computed; gate_w columns are permuted per core so each core's
    own experts sit in columns 0/1 (keeps the program SPMD-identical).
  - The shared expert is sharded over its intermediate dim (4096 -> 512
    per core); each core emits a partial z. Unshard = sum over cores
    (contraction-dim sharding) on the host, plus scatter-add of the
    weighted routed-expert outputs.

The FFN datapath runs in bf16 (measured equal accuracy to fp32r on this
HW -- both round operands to ~bf16 -- at half the DMA traffic); PSUM
accumulation and all outputs are fp32. The gate path is exact fp32.
"""

import os
import numpy as np
import ml_dtypes
from contextlib import ExitStack

import concourse.bacc as bacc
import concourse.mybir as mybir
import concourse.tile as tile
from concourse.bass_utils import run_bass_kernel_spmd

F32 = mybir.dt.float32
BF16 = mybir.dt.float16
NPBF = np.float16
AF = mybir.ActivationFunctionType
ALU = mybir.AluOpType
AX = mybir.AxisListType

B, S, D = 16, 196, 768
T = B * S            # 3136 tokens
E, TOPK = 16, 2
I = 2048             # routed expert intermediate
SI = 4096            # shared expert intermediate
NCORES, EPC = 8, 2   # cores, experts per core
KD = D // 128        # 6   k-tiles over D
KI = I // 128        # 16  k-tiles over I
SIC = SI // NCORES   # 512 shared-inter slice per core
KSI = SIC // 128     # 4
NT = 7
TCH = T // NT        # 448 token chunk for the shared expert

_CACHE = {}
LAST_RESULT = None


def build_bass(CS):
    """Build the SPMD Bass program. CS = (C0, C1): per-slot routed-expert
    capacities (slot 1 carries the high-count experts). Identical across
    cores, so the program stays SPMD."""
    CS = tuple(CS)
    NM2S = [(c + 127) // 128 for c in CS]
    NM2MAX = max(NM2S)

    nc = bacc.Bacc("TRN2", target_bir_lowering=False, debug=False,
                   num_devices=NCORES)

    xTt = nc.dram_tensor("xTt", [NT, 128, KD, TCH], BF16, kind="ExternalInput")
    xgTs = [nc.dram_tensor(f"xgT{e}", [128, KD, CS[e]], BF16,
                           kind="ExternalInput") for e in range(EPC)]
    xg32s = [nc.dram_tensor(f"xg32_{e}", [128, KD, CS[e]], F32,
                            kind="ExternalInput") for e in range(EPC)]
    gwp = nc.dram_tensor("gwp", [128, KD, E], F32, kind="ExternalInput")
    w13 = nc.dram_tensor("w13", [EPC, KI, 128, 2, KD, 128], BF16,
                         kind="ExternalInput")
    w2r = nc.dram_tensor("w2r", [EPC, 128, KI, D], BF16,
                         kind="ExternalInput")
    ws1t = nc.dram_tensor("ws1t", [128, KD, SIC], BF16, kind="ExternalInput")
    ws3t = nc.dram_tensor("ws3t", [128, KD, SIC], BF16, kind="ExternalInput")
    ws2t = nc.dram_tensor("ws2t", [128, KSI, D], BF16, kind="ExternalInput")

    og = nc.dram_tensor("og", [EPC, NM2MAX, 128, D], F32,
                        kind="ExternalOutput")
    zt = nc.dram_tensor("zt", [KD, 128, NT, TCH], F32, kind="ExternalOutput")

    with tile.TileContext(nc) as tc, ExitStack() as ctx:
        cpool = ctx.enter_context(tc.tile_pool(name="consts", bufs=1))
        xpool = ctx.enter_context(tc.tile_pool(name="xstream", bufs=2))
        gxpool = ctx.enter_context(tc.tile_pool(name="xg", bufs=2))
        g32pool = ctx.enter_context(tc.tile_pool(name="xg32p", bufs=2))
        hspool = ctx.enter_context(tc.tile_pool(name="hs", bufs=2))
        hpool = ctx.enter_context(tc.tile_pool(name="h", bufs=2))
        upool = ctx.enter_context(tc.tile_pool(name="u", bufs=3))
        wpool13 = ctx.enter_context(tc.tile_pool(name="w13s", bufs=6))
        w2rpool = ctx.enter_context(tc.tile_pool(name="w2r", bufs=2))
        ospool = ctx.enter_context(tc.tile_pool(name="osb", bufs=3))
        zpool = ctx.enter_context(tc.tile_pool(name="zsb", bufs=6))
        spool = ctx.enter_context(tc.tile_pool(name="smax", bufs=8))
        wgtpool = ctx.enter_context(tc.tile_pool(name="wgt", bufs=2))
        acc = ctx.enter_context(tc.tile_pool(name="acc", bufs=5, space="PSUM"))
        opsum = ctx.enter_context(tc.tile_pool(name="opsum", bufs=3,
                                               space="PSUM"))

        # resident-constant tiles; DMAs are emitted later, at points chosen so
        # queue order matches each tile's first-use deadline
        gw_sb = cpool.tile([128, KD, E], F32, tag="gw")
        ws1_sb = cpool.tile([128, KD, SIC], BF16, tag="ws1")
        ws3_sb = cpool.tile([128, KD, SIC], BF16, tag="ws3")
        ws2_sb = cpool.tile([128, KSI, D], BF16, tag="ws2")

        def emit_const_loads():
            nc.sync.dma_start(ws1_sb[:, 0:3], ws1t[:, 0:3])
            nc.sync.dma_start(ws1_sb[:, 3:6], ws1t[:, 3:6])
            nc.sync.dma_start(ws3_sb[:, 0:3], ws3t[:, 0:3])
            nc.sync.dma_start(ws3_sb[:, 3:6], ws3t[:, 3:6])
            nc.sync.dma_start(ws2_sb[:, 0:2], ws2t[:, 0:2])
            nc.sync.dma_start(ws2_sb[:, 2:4], ws2t[:, 2:4])

        def emit_shared_nt(nt):
            xt_sb = xpool.tile([128, KD, TCH], BF16, tag="xt")
            nc.sync.dma_start(xt_sb[:], xTt[nt])
            hs_sb = hspool.tile([128, KSI, TCH], BF16, tag="hs")
            for mi in range(KSI):
                ps1 = acc.tile([128, TCH], F32, tag="acc")
                for k in range(KD):
                    nc.tensor.matmul(ps1[:], ws1_sb[:, k, mi * 128:(mi + 1) * 128],
                                     xt_sb[:, k, :],
                                     start=(k == 0), stop=(k == KD - 1))
                u1 = upool.tile([128, TCH], F32, tag="u")
                nc.scalar.activation(u1[:], ps1[:], AF.Silu)
                ps3 = acc.tile([128, TCH], F32, tag="acc")
                for k in range(KD):
                    nc.tensor.matmul(ps3[:], ws3_sb[:, k, mi * 128:(mi + 1) * 128],
                                     xt_sb[:, k, :],
                                     start=(k == 0), stop=(k == KD - 1))
                nc.vector.tensor_mul(hs_sb[:, mi, :], u1[:], ps3[:])
            for m in range(KD):
                psz = acc.tile([128, TCH], F32, tag="acc")
                for ki in range(KSI):
                    nc.tensor.matmul(psz[:],
                                     ws2_sb[:, ki, m * 128:(m + 1) * 128],
                                     hs_sb[:, ki, :],
                                     start=(ki == 0), stop=(ki == KSI - 1))
                zsb = zpool.tile([128, TCH], F32, tag="z")
                nc.any.tensor_copy(out=zsb[:], in_=psz[:])
                nc.sync.dma_start(zt[m, :, nt, :], zsb[:])

        def emit_gate(e, x32_sb):
            # gate: combine weight per token slot (token on partitions), fp32
            Ce = CS[e]
            wgt_sb = wgtpool.tile([128, NM2S[e]], F32, tag="wgt",
                                  name=f"wgt_{e}")
            for m2 in range(NM2S[e]):
                pm = min(128, Ce - m2 * 128)
                psl = acc.tile([128, E], F32, tag="acc")
                for k in range(KD):
                    nc.tensor.matmul(psl[:pm], x32_sb[:, k, m2 * 128:m2 * 128 + pm],
                                     gw_sb[:, k, :],
                                     start=(k == 0), stop=(k == KD - 1))
                nmax = spool.tile([128, 1], F32, tag="nmax")
                nc.vector.reduce_max(out=nmax[:pm], in_=psl[:pm], axis=AX.X,
                                     negate=True)
                esc = spool.tile([128, E], F32, tag="esc")
                nc.scalar.activation(esc[:pm], psl[:pm], AF.Exp,
                                     bias=nmax[:pm])
                ssum = spool.tile([128, 1], F32, tag="ssum")
                nc.vector.reduce_sum(out=ssum[:pm], in_=esc[:pm], axis=AX.X)
                rec = spool.tile([128, 1], F32, tag="rec")
                nc.vector.reciprocal(rec[:pm], ssum[:pm])
                nc.vector.tensor_mul(wgt_sb[:pm, m2:m2 + 1],
                                     esc[:pm, e:e + 1], rec[:pm])
            return wgt_sb

        def emit_expert_ffn(e, xg_sb):
            # FFN: h.T tiles [inter-part, C] in fp16
            Ce = CS[e]
            cchunks = [(s, min(512, Ce - s)) for s in range(0, Ce, 512)]
            h_sb = hpool.tile([128, KI, Ce], BF16, tag="h", name=f"h_{e}")
            for mi in range(KI):
                wt = wpool13.tile([128, 2, KD, 128], BF16, tag="w13")
                early = (e == 0 and mi < 2)
                eng = nc.gpsimd if early else nc.sync
                nsplit = 3 if mi < 2 else 1
                for b in range(2):
                    step = (KD + nsplit - 1) // nsplit
                    for k0 in range(0, KD, step):
                        k1 = min(KD, k0 + step)
                        eng.dma_start(wt[:, b:b + 1, k0:k1],
                                      w13[e, mi, :, b:b + 1, k0:k1])
                for (cs, cn) in cchunks:
                    ps1 = acc.tile([128, cn], F32, tag="acc")
                    for k in range(KD):
                        nc.tensor.matmul(ps1[:], wt[:, 0, k, :],
                                         xg_sb[:, k, cs:cs + cn],
                                         start=(k == 0), stop=(k == KD - 1))
                    u1 = upool.tile([128, cn], F32, tag="u")
                    nc.scalar.activation(u1[:], ps1[:], AF.Silu)
                    ps3 = acc.tile([128, cn], F32, tag="acc")
                    for k in range(KD):
                        nc.tensor.matmul(ps3[:], wt[:, 1, k, :],
                                         xg_sb[:, k, cs:cs + cn],
                                         start=(k == 0), stop=(k == KD - 1))
                    nc.vector.tensor_mul(h_sb[:, mi, cs:cs + cn], u1[:],
                                         ps3[:])
            return h_sb

        def emit_expert_o(e, h_sb, w2_sb, wgt_sb):
            # o = (h.T).T @ w2.T, tokens on partitions; scale by gate weight
            Ce = CS[e]
            for m2 in range(NM2S[e]):
                pm = min(128, Ce - m2 * 128)
                osb = ospool.tile([128, D], F32, tag="osb")
                for (n, nsz) in ((0, 512), (1, 256)):
                    pos = opsum.tile([128, nsz], F32, tag="o")
                    for ki in range(KI):
                        nc.tensor.matmul(
                            pos[:pm],
                            h_sb[:, ki, m2 * 128:m2 * 128 + pm],
                            w2_sb[:, ki, n * 512:n * 512 + nsz],
                            start=(ki == 0), stop=(ki == KI - 1))
                    nc.scalar.activation(osb[:pm, n * 512:n * 512 + nsz],
                                         pos[:pm], AF.Copy,
                                         scale=wgt_sb[:pm, m2:m2 + 1])
                    # ship each half as soon as its scale-copy lands
                    for d0 in range(n * 512, n * 512 + nsz, 256):
                        nc.sync.dma_start(og[e, m2, :pm, d0:d0 + 256],
                                          osb[:pm, d0:d0 + 256])

        # --- emission order: routed experts first (their weight streams
        # overlap compute), shared expert last (all-resident weights, so the
        # kernel tail has no DMA-dependent matmuls). DMAs are emitted in
        # first-use-deadline order so HWDGE queue FIFOs prioritize the
        # startup-critical bytes.
        def load_xg(e, early=False):
            eng = nc.gpsimd if early else nc.sync
            t = gxpool.tile([128, KD, CS[e]], BF16, tag="xg", name=f"xg_{e}")
            for k in range(KD):
                eng.dma_start(t[:, k], xgTs[e][:, k])
            return t

        def load_x32(e):
            t = g32pool.tile([128, KD, CS[e]], F32, tag="x32",
                             name=f"x32_{e}")
            nc.sync.dma_start(t[:, 0:3], xg32s[e][:, 0:3])
            nc.sync.dma_start(t[:, 3:6], xg32s[e][:, 3:6])
            return t

        xg0 = load_xg(0, early=True)
        h0 = emit_expert_ffn(0, xg0)

        nc.sync.dma_start(gw_sb[:], gwp[:])
        x320 = load_x32(0)
        wgt0 = emit_gate(0, x320)
        x321 = load_x32(1)
        wgt1 = emit_gate(1, x321)
        xg1 = load_xg(1)

        # resident w2 for both experts; queued behind the gate/xg loads,
        # needed ~80us in
        w2_sbs = {}
        for e in range(EPC):
            w2_sbs[e] = w2rpool.tile([128, KI, D], BF16, tag="w2r",
                                     name=f"w2r_{e}")
            for kg in range(0, KI, 4):
                nc.sync.dma_start(w2_sbs[e][:, kg:kg + 4], w2r[e, :, kg:kg + 4])
        emit_const_loads()

        h1 = emit_expert_ffn(1, xg1)
        emit_expert_o(0, h0, w2_sbs[0], wgt0)
        for nt in range(NT):
            emit_shared_nt(nt)
        # expert-1 combine last: all its inputs are resident long before, so
        # the scheduler can use it to fill the shared-phase tail stalls
        emit_expert_o(1, h1, w2_sbs[1], wgt1)

    nc.finalize()
    return nc


def _kd_tile(A, dtype):
    """[N, D] -> [128, KD, N] with out[p, ko, n] = A[n, ko*128+p]."""
    N = A.shape[0]
    return np.ascontiguousarray(
        A.T.reshape(KD, 128, N).transpose(1, 0, 2)).astype(dtype)


def kernel(x, gate_w, w1, w2, w3, ws1, ws2, ws3):
    global LAST_RESULT
    x = np.asarray(x, np.float32)
    gate_w = np.asarray(gate_w, np.float32)
    w1 = np.asarray(w1, np.float32)
    w2 = np.asarray(w2, np.float32)
    w3 = np.asarray(w3, np.float32)
    ws1 = np.asarray(ws1, np.float32)
    ws2 = np.asarray(ws2, np.float32)
    ws3 = np.asarray(ws3, np.float32)

    xt = x.reshape(T, D)

    # --- host routing (dispatch == sharding): which tokens go to which core
    logits = xt.astype(np.float64) @ gate_w.T.astype(np.float64)
    order = np.argsort(-logits, axis=1, kind="stable")
    top2 = order[:, :TOPK]
    idx = [np.nonzero((top2 == e).any(axis=1))[0] for e in range(E)]
    cnt = np.array([len(ix) for ix in idx])

    # assign the 8 largest-count experts to slot 1, smallest to slot 0, so
    # the (SPMD-uniform) per-slot capacities hug the actual counts
    byc = np.argsort(-cnt, kind="stable")
    slot1, slot0 = byc[:NCORES], byc[NCORES:]
    pad16 = lambda v: max(128, int(np.ceil(v / 16.0)) * 16)
    CS = (pad16(cnt[slot0].max()), pad16(cnt[slot1].max()))
    assign = [(int(slot0[c]), int(slot1[c])) for c in range(NCORES)]

    if CS not in _CACHE:
        _CACHE[CS] = build_bass(CS)
    nc = _CACHE[CS]
    NM2MAX = (CS[1] + 127) // 128

    # --- per-core input marshalling
    xt_kd = _kd_tile(xt, NPBF)                             # [128, KD, T]
    xTt = np.ascontiguousarray(
        xt_kd.reshape(128, KD, NT, TCH).transpose(2, 0, 1, 3))

    in_maps = []
    for c in range(NCORES):
        e0, e1 = assign[c]
        perm = [e0, e1] + [e for e in range(E) if e not in (e0, e1)]
        gwp = _kd_tile(gate_w[perm], np.float32)           # [128, KD, E]

        xg, xg32 = {}, {}
        for el, eg in enumerate((e0, e1)):
            ixp = np.zeros(CS[el], np.int64)
            ixp[:cnt[eg]] = idx[eg]
            xg32[el] = _kd_tile(xt[ixp], np.float32)
            xg[el] = xg32[el].astype(NPBF)

        w13 = np.empty((EPC, KI, 128, 2, KD, 128), NPBF)
        w2rs = np.empty((EPC, 128, KI, D), NPBF)
        for el, eg in enumerate((e0, e1)):
            for b, wb in enumerate((w1, w3)):
                w13[el, :, :, b] = (_kd_tile(wb[eg], NPBF)  # [128, KD, I]
                                    .reshape(128, KD, KI, 128)
                                    .transpose(2, 0, 1, 3))
            w2rs[el] = np.ascontiguousarray(w2[eg].T).astype(NPBF).reshape(
                KI, 128, D).transpose(1, 0, 2)

        sl = slice(c * SIC, (c + 1) * SIC)
        ws1t = _kd_tile(ws1[sl], NPBF)                     # [128, KD, SIC]
        ws3t = _kd_tile(ws3[sl], NPBF)
        ws2t = np.ascontiguousarray(
            ws2[:, sl].T.reshape(KSI, 128, D).transpose(1, 0, 2)).astype(NPBF)

        in_maps.append({
            "xTt": xTt, "xgT0": xg[0], "xgT1": xg[1],
            "xg32_0": xg32[0], "xg32_1": xg32[1], "gwp": gwp, "w13": w13,
            "w2r": w2rs,
            "ws1t": ws1t, "ws3t": ws3t, "ws2t": ws2t,
        })

    trace = bool(int(os.environ.get("MOE_TRACE", "0")))
    res = run_bass_kernel_spmd(nc, in_maps, core_ids=list(range(NCORES)),
                               trace=trace)
    LAST_RESULT = res

    # --- unshard: sum shared-expert partials, scatter-add routed outputs
    zsum = np.zeros((KD, 128, NT, TCH), np.float64)
    for c in range(NCORES):
        zsum += res.results[c]["zt"]
    y = np.ascontiguousarray(zsum.reshape(D, T).T)         # [T, D] float64

    for c in range(NCORES):
        ogc = res.results[c]["og"]                      # [EPC, NM2MAX, 128, D]
        for el in range(EPC):
            eg = assign[c][el]
            rows = ogc[el].reshape(NM2MAX * 128, D)[:cnt[eg]]
            y[idx[eg]] += rows

    return y.astype(np.float32).reshape(B, S, D)


# revision 39
# speedup vs baseline: 1.0630x; 1.0630x over previous
"""MoE (16 routed experts, top-2, + shared expert) for 8 Trainium2 cores.

Sharding strategy (expert-parallel, per the hint):
  - Each core owns 2 routed experts. The host computes the gate routing
    (dispatch = sharding: selecting which token rows go to which core) and
    ships each core a gathered, capacity-padded token block per expert.
  - The gate combine WEIGHTS are recomputed on device (fp32 matmul +
    softmax on the gathered tokens) so all output values are
    device-computed; gate_w columns are permuted per core so each core's
    own experts sit in columns 0/1 (keeps the program SPMD-identical).
  - The shared expert is sharded over its intermediate dim (4096 -> 512
    per core); each core emits a partial z. Unshard = sum over cores
    (contraction-dim sharding) on the host, plus scatter-add of the
    weighted routed-expert outputs.

The FFN datapath runs in bf16 (measured equal accuracy to fp32r on this
HW -- both round operands to ~bf16 -- at half the DMA traffic); PSUM
accumulation and all outputs are fp32. The gate path is exact fp32.
"""

import os
import numpy as np
import ml_dtypes
from contextlib import ExitStack

import concourse.bacc as bacc
import concourse.mybir as mybir
import concourse.tile as tile
from concourse.bass_utils import run_bass_kernel_spmd

F32 = mybir.dt.float32
BF16 = mybir.dt.float16
NPBF = np.float16
AF = mybir.ActivationFunctionType
ALU = mybir.AluOpType
AX = mybir.AxisListType

B, S, D = 16, 196, 768
T = B * S            # 3136 tokens
E, TOPK = 16, 2
I = 2048             # routed expert intermediate
SI = 4096            # shared expert intermediate
NCORES, EPC = 8, 2   # cores, experts per core
KD = D // 128        # 6   k-tiles over D
KI = I // 128        # 16  k-tiles over I
SIC = SI // NCORES   # 512 shared-inter slice per core
KSI = SIC // 128     # 4
NT = 7
TCH = T // NT        # 448 token chunk for the shared expert

_CACHE = {}
LAST_RESULT = None


def build_bass(CS):
    """Build the SPMD Bass program. CS = (C0, C1): per-slot routed-expert
    capacities (slot 1 carries the high-count experts). Identical across
    cores, so the program stays SPMD."""
    CS = tuple(CS)
    NM2S = [(c + 127) // 128 for c in CS]
    NM2MAX = max(NM2S)

    nc = bacc.Bacc("TRN2", target_bir_lowering=False, debug=False,
                   num_devices=NCORES)

    xTt = nc.dram_tensor("xTt", [NT, 128, KD, TCH], BF16, kind="ExternalInput")
    xgTs = [nc.dram_tensor(f"xgT{e}", [128, KD, CS[e]], BF16,
                           kind="ExternalInput") for e in range(EPC)]
    xg32s = [nc.dram_tensor(f"xg32_{e}", [128, KD, CS[e]], F32,
                            kind="ExternalInput") for e in range(EPC)]
    gwp = nc.dram_tensor("gwp", [128, KD, E], F32, kind="ExternalInput")
    w13 = nc.dram_tensor("w13", [EPC, KI, 128, 2, KD, 128], BF16,
                         kind="ExternalInput")
    w2r = nc.dram_tensor("w2r", [EPC, 128, KI, D], BF16,
                         kind="ExternalInput")
    ws1t = nc.dram_tensor("ws1t", [128, KD, SIC], BF16, kind="ExternalInput")
    ws3t = nc.dram_tensor("ws3t", [128, KD, SIC], BF16, kind="ExternalInput")
    ws2t = nc.dram_tensor("ws2t", [128, KSI, D], BF16, kind="ExternalInput")

    og = nc.dram_tensor("og", [EPC, NM2MAX, 128, D], F32,
                        kind="ExternalOutput")
    zt = nc.dram_tensor("zt", [KD, 128, NT, TCH], F32, kind="ExternalOutput")

    with tile.TileContext(nc) as tc, ExitStack() as ctx:
        cpool = ctx.enter_context(tc.tile_pool(name="consts", bufs=1))
        xpool = ctx.enter_context(tc.tile_pool(name="xstream", bufs=2))
        gxpool = ctx.enter_context(tc.tile_pool(name="xg", bufs=2))
        g32pool = ctx.enter_context(tc.tile_pool(name="xg32p", bufs=2))
        hspool = ctx.enter_context(tc.tile_pool(name="hs", bufs=2))
        hpool = ctx.enter_context(tc.tile_pool(name="h", bufs=2))
        upool = ctx.enter_context(tc.tile_pool(name="u", bufs=3))
        wpool13 = ctx.enter_context(tc.tile_pool(name="w13s", bufs=6))
        w2rpool = ctx.enter_context(tc.tile_pool(name="w2r", bufs=2))
        ospool = ctx.enter_context(tc.tile_pool(name="osb", bufs=3))
        zpool = ctx.enter_context(tc.tile_pool(name="zsb", bufs=6))
        spool = ctx.enter_context(tc.tile_pool(name="smax", bufs=8))
        wgtpool = ctx.enter_context(tc.tile_pool(name="wgt", bufs=2))
        acc = ctx.enter_context(tc.tile_pool(name="acc", bufs=4, space="PSUM"))
        opsum = ctx.enter_context(tc.tile_pool(name="opsum", bufs=4,
                                               space="PSUM"))

        # resident-constant tiles; DMAs are emitted later, at points chosen so
        # queue order matches each tile's first-use deadline
        gw_sb = cpool.tile([128, KD, E], F32, tag="gw")
        ws1_sb = cpool.tile([128, KD, SIC], BF16, tag="ws1")
        ws3_sb = cpool.tile([128, KD, SIC], BF16, tag="ws3")
        ws2_sb = cpool.tile([128, KSI, D], BF16, tag="ws2")

        def emit_const_loads():
            nc.sync.dma_start(ws1_sb[:, 0:3], ws1t[:, 0:3])
            nc.sync.dma_start(ws1_sb[:, 3:6], ws1t[:, 3:6])
            nc.sync.dma_start(ws3_sb[:, 0:3], ws3t[:, 0:3])
            nc.sync.dma_start(ws3_sb[:, 3:6], ws3t[:, 3:6])
            nc.sync.dma_start(ws2_sb[:, 0:2], ws2t[:, 0:2])
            nc.sync.dma_start(ws2_sb[:, 2:4], ws2t[:, 2:4])

        def emit_shared_nt(nt):
            xt_sb = xpool.tile([128, KD, TCH], BF16, tag="xt")
            nc.sync.dma_start(xt_sb[:], xTt[nt])
            hs_sb = hspool.tile([128, KSI, TCH], BF16, tag="hs")
            for mi in range(KSI):
                ps1 = acc.tile([128, TCH], F32, tag="acc")
                for k in range(KD):
                    nc.tensor.matmul(ps1[:], ws1_sb[:, k, mi * 128:(mi + 1) * 128],
                                     xt_sb[:, k, :],
                                     start=(k == 0), stop=(k == KD - 1))
                u1 = upool.tile([128, TCH], F32, tag="u")
                nc.scalar.activation(u1[:], ps1[:], AF.Silu)
                ps3 = acc.tile([128, TCH], F32, tag="acc")
                for k in range(KD):
                    nc.tensor.matmul(ps3[:], ws3_sb[:, k, mi * 128:(mi + 1) * 128],
                                     xt_sb[:, k, :],
                                     start=(k == 0), stop=(k == KD - 1))
                nc.vector.tensor_mul(hs_sb[:, mi, :], u1[:], ps3[:])
            for m in range(KD):
                psz = acc.tile([128, TCH], F32, tag="acc")
                for ki in range(KSI):
                    nc.tensor.matmul(psz[:],
                                     ws2_sb[:, ki, m * 128:(m + 1) * 128],
                                     hs_sb[:, ki, :],
                                     start=(ki == 0), stop=(ki == KSI - 1))
                zsb = zpool.tile([128, TCH], F32, tag="z")
                nc.any.tensor_copy(out=zsb[:], in_=psz[:])
                nc.sync.dma_start(zt[m, :, nt, :], zsb[:])

        def emit_gate(e, x32_sb):
            # gate: combine weight per token slot (token on partitions), fp32
            Ce = CS[e]
            wgt_sb = wgtpool.tile([128, NM2S[e]], F32, tag="wgt",
                                  name=f"wgt_{e}")
            for m2 in range(NM2S[e]):
                pm = min(128, Ce - m2 * 128)
                psl = acc.tile([128, E], F32, tag="acc")
                for k in range(KD):
                    nc.tensor.matmul(psl[:pm], x32_sb[:, k, m2 * 128:m2 * 128 + pm],
                                     gw_sb[:, k, :],
                                     start=(k == 0), stop=(k == KD - 1))
                nmax = spool.tile([128, 1], F32, tag="nmax")
                nc.vector.reduce_max(out=nmax[:pm], in_=psl[:pm], axis=AX.X,
                                     negate=True)
                esc = spool.tile([128, E], F32, tag="esc")
                nc.scalar.activation(esc[:pm], psl[:pm], AF.Exp,
                                     bias=nmax[:pm])
                ssum = spool.tile([128, 1], F32, tag="ssum")
                nc.vector.reduce_sum(out=ssum[:pm], in_=esc[:pm], axis=AX.X)
                rec = spool.tile([128, 1], F32, tag="rec")
                nc.vector.reciprocal(rec[:pm], ssum[:pm])
                nc.vector.tensor_mul(wgt_sb[:pm, m2:m2 + 1],
                                     esc[:pm, e:e + 1], rec[:pm])
            return wgt_sb

        def emit_expert_ffn(e, xg_sb):
            # FFN: h.T tiles [inter-part, C] in fp16
            Ce = CS[e]
            cchunks = [(s, min(512, Ce - s)) for s in range(0, Ce, 512)]
            h_sb = hpool.tile([128, KI, Ce], BF16, tag="h", name=f"h_{e}")
            for mi in range(KI):
                wt = wpool13.tile([128, 2, KD, 128], BF16, tag="w13")
                eng = nc.sync
                nsplit = 3 if mi < 2 else 1
                for b in range(2):
                    step = (KD + nsplit - 1) // nsplit
                    for k0 in range(0, KD, step):
                        k1 = min(KD, k0 + step)
                        eng.dma_start(wt[:, b:b + 1, k0:k1],
                                      w13[e, mi, :, b:b + 1, k0:k1])
                for (cs, cn) in cchunks:
                    ps1 = acc.tile([128, cn], F32, tag="acc")
                    for k in range(KD):
                        nc.tensor.matmul(ps1[:], wt[:, 0, k, :],
                                         xg_sb[:, k, cs:cs + cn],
                                         start=(k == 0), stop=(k == KD - 1))
                    u1 = upool.tile([128, cn], F32, tag="u")
                    nc.scalar.activation(u1[:], ps1[:], AF.Silu)
                    ps3 = acc.tile([128, cn], F32, tag="acc")
                    for k in range(KD):
                        nc.tensor.matmul(ps3[:], wt[:, 1, k, :],
                                         xg_sb[:, k, cs:cs + cn],
                                         start=(k == 0), stop=(k == KD - 1))
                    nc.vector.tensor_mul(h_sb[:, mi, cs:cs + cn], u1[:],
                                         ps3[:])
            return h_sb

        def emit_expert_o(e, h_sb, w2_sb, wgt_sb):
            # o = (h.T).T @ w2.T, tokens on partitions; scale by gate weight
            Ce = CS[e]
            for m2 in range(NM2S[e]):
                pm = min(128, Ce - m2 * 128)
                osb = ospool.tile([128, D], F32, tag="osb")
                for (n, nsz) in ((0, 512), (1, 256)):
                    pos = opsum.tile([128, nsz], F32, tag="o")
                    for ki in range(KI):
                        nc.tensor.matmul(
                            pos[:pm],
                            h_sb[:, ki, m2 * 128:m2 * 128 + pm],
                            w2_sb[:, ki, n * 512:n * 512 + nsz],
                            start=(ki == 0), stop=(ki == KI - 1))
                    nc.scalar.activation(osb[:pm, n * 512:n * 512 + nsz],
                                         pos[:pm], AF.Copy,
                                         scale=wgt_sb[:pm, m2:m2 + 1])
                    # ship each half as soon as its scale-copy lands
                    for d0 in range(n * 512, n * 512 + nsz, 256):
                        nc.sync.dma_start(og[e, m2, :pm, d0:d0 + 256],
                                          osb[:pm, d0:d0 + 256])

        # --- emission order: routed experts first (their weight streams
        # overlap compute), shared expert last (all-resident weights, so the
        # kernel tail has no DMA-dependent matmuls). DMAs are emitted in
        # first-use-deadline order so HWDGE queue FIFOs prioritize the
        # startup-critical bytes.
        def load_xg(e, early=False):
            eng = nc.gpsimd if early else nc.sync
            t = gxpool.tile([128, KD, CS[e]], BF16, tag="xg", name=f"xg_{e}")
            for k in range(KD):
                eng.dma_start(t[:, k], xgTs[e][:, k])
            return t

        def load_x32(e):
            t = g32pool.tile([128, KD, CS[e]], F32, tag="x32",
                             name=f"x32_{e}")
            nc.sync.dma_start(t[:, 0:3], xg32s[e][:, 0:3])
            nc.sync.dma_start(t[:, 3:6], xg32s[e][:, 3:6])
            return t

        xg0 = load_xg(0, early=True)
        h0 = emit_expert_ffn(0, xg0)

        nc.sync.dma_start(gw_sb[:], gwp[:])
        x320 = load_x32(0)
        wgt0 = emit_gate(0, x320)
        x321 = load_x32(1)
        wgt1 = emit_gate(1, x321)
        xg1 = load_xg(1)

        # resident w2 for both experts; queued behind the gate/xg loads,
        # needed ~80us in
        w2_sbs = {}
        for e in range(EPC):
            w2_sbs[e] = w2rpool.tile([128, KI, D], BF16, tag="w2r",
                                     name=f"w2r_{e}")
            for kg in range(0, KI, 4):
                nc.sync.dma_start(w2_sbs[e][:, kg:kg + 4], w2r[e, :, kg:kg + 4])
        emit_const_loads()

        h1 = emit_expert_ffn(1, xg1)
        emit_expert_o(0, h0, w2_sbs[0], wgt0)
        for nt in range(NT):
            emit_shared_nt(nt)
        # expert-1 combine last: all its inputs are resident long before, so
        # the scheduler can use it to fill the shared-phase tail stalls
        emit_expert_o(1, h1, w2_sbs[1], wgt1)

    nc.finalize()
    return nc


def _kd_tile(A, dtype):
    """[N, D] -> [128, KD, N] with out[p, ko, n] = A[n, ko*128+p]."""
    N = A.shape[0]
    return np.ascontiguousarray(
        A.T.reshape(KD, 128, N).transpose(1, 0, 2)).astype(dtype)


def kernel(x, gate_w, w1, w2, w3, ws1, ws2, ws3):
    global LAST_RESULT
    x = np.asarray(x, np.float32)
    gate_w = np.asarray(gate_w, np.float32)
    w1 = np.asarray(w1, np.float32)
    w2 = np.asarray(w2, np.float32)
    w3 = np.asarray(w3, np.float32)
    ws1 = np.asarray(ws1, np.float32)
    ws2 = np.asarray(ws2, np.float32)
    ws3 = np.asarray(ws3, np.float32)

    xt = x.reshape(T, D)

    # --- host routing (dispatch == sharding): which tokens go to which core
    logits = xt.astype(np.float64) @ gate_w.T.astype(np.float64)
    order = np.argsort(-logits, axis=1, kind="stable")
    top2 = order[:, :TOPK]
    idx = [np.nonzero((top2 == e).any(axis=1))[0] for e in range(E)]
    cnt = np.array([len(ix) for ix in idx])

    # assign the 8 largest-count experts to slot 1, smallest to slot 0, so
    # the (SPMD-uniform) per-slot capacities hug the actual counts
    byc = np.argsort(-cnt, kind="stable")
    slot1, slot0 = byc[:NCORES], byc[NCORES:]
    pad16 = lambda v: max(128, int(np.ceil(v / 16.0)) * 16)
    CS = (pad16(cnt[slot0].max()), pad16(cnt[slot1].max()))
    assign = [(int(slot0[c]), int(slot1[c])) for c in range(NCORES)]

    if CS not in _CACHE:
        _CACHE[CS] = build_bass(CS)
    nc = _CACHE[CS]
    NM2MAX = (CS[1] + 127) // 128

    # --- per-core input marshalling
    xt_kd = _kd_tile(xt, NPBF)                             # [128, KD, T]
    xTt = np.ascontiguousarray(
        xt_kd.reshape(128, KD, NT, TCH).transpose(2, 0, 1, 3))

    in_maps = []
    for c in range(NCORES):
        e0, e1 = assign[c]
        perm = [e0, e1] + [e for e in range(E) if e not in (e0, e1)]
        gwp = _kd_tile(gate_w[perm], np.float32)           # [128, KD, E]

        xg, xg32 = {}, {}
        for el, eg in enumerate((e0, e1)):
            ixp = np.zeros(CS[el], np.int64)
            ixp[:cnt[eg]] = idx[eg]
            xg32[el] = _kd_tile(xt[ixp], np.float32)
            xg[el] = xg32[el].astype(NPBF)

        w13 = np.empty((EPC, KI, 128, 2, KD, 128), NPBF)
        w2rs = np.empty((EPC, 128, KI, D), NPBF)
        for el, eg in enumerate((e0, e1)):
            for b, wb in enumerate((w1, w3)):
                w13[el, :, :, b] = (_kd_tile(wb[eg], NPBF)  # [128, KD, I]
                                    .reshape(128, KD, KI, 128)
                                    .transpose(2, 0, 1, 3))
            w2rs[el] = np.ascontiguousarray(w2[eg].T).astype(NPBF).reshape(
                KI, 128, D).transpose(1, 0, 2)

        sl = slice(c * SIC, (c + 1) * SIC)
        ws1t = _kd_tile(ws1[sl], NPBF)                     # [128, KD, SIC]
        ws3t = _kd_tile(ws3[sl], NPBF)
        ws2t = np.ascontiguousarray(
            ws2[:, sl].T.reshape(KSI, 128, D).transpose(1, 0, 2)).astype(NPBF)

        in_maps.append({
            "xTt": xTt, "xgT0": xg[0], "xgT1": xg[1],
            "xg32_0": xg32[0], "xg32_1": xg32[1], "gwp": gwp, "w13": w13,
            "w2r": w2rs,
            "ws1t": ws1t, "ws3t": ws3t, "ws2t": ws2t,
        })

    trace = bool(int(os.environ.get("MOE_TRACE", "0")))
    res = run_bass_kernel_spmd(nc, in_maps, core_ids=list(range(NCORES)),
                               trace=trace)
    LAST_RESULT = res

    # --- unshard: sum shared-expert partials, scatter-add routed outputs
    zsum = np.zeros((KD, 128, NT, TCH), np.float64)
    for c in range(NCORES):
        zsum += res.results[c]["zt"]
    y = np.ascontiguousarray(zsum.reshape(D, T).T)         # [T, D] float64

    for c in range(NCORES):
        ogc = res.results[c]["og"]                      # [EPC, NM2MAX, 128, D]
        for el in range(EPC):
            eg = assign[c][el]
            rows = ogc[el].reshape(NM2MAX * 128, D)[:cnt[eg]]
            y[idx[eg]] += rows

    return y.astype(np.float32).reshape(B, S, D)
